# revision 18
# baseline (speedup 1.0000x reference)
"""GAT (2-layer) Trainium2 Bass kernel — 8-core SPMD.

Strategy (dst-partitioned, degree-regular):
  - Host: nodes bucketed by in-degree (padded to pow2 classes d in {1..128}),
    dealt round-robin to 8 cores, padded so all cores share one class profile.
    Node "rank" space (per core) replaces node ids everywhere on device.
  - Table trick: gather table row = feat @ (W·H·D) where H = Householder
    mapping attn_l -> e_col, D scales that col by ||attn_l||. So the per-edge
    gather brings both the features (rotated) and el = feat@attn_l in ONE row.
    er = feat@(W@attn_r) computed per node at table build.
  - Edge phase: per 128-node window of one degree class d: gather k=d chunks
    of 128 edge-rows (indirect DMA, edge e at [e%128, e//128]); er broadcast
    per window via a step-0 strided DMA; w = exp(leakyrelu(el+er));
    segment-sum via ONE constant matmul per chunk (S_d = I_{128/d} (x) 1_d),
    PSUM node-major [128, 65] = [sum w*feat' | sum w].
  - Flush per window: transpose (PE), unrotate via (H D^-1) matmul, divide by
    s, +bias, (ELU) -> h tile; layer-2 table built per tile on the fly.
  - AllGather (8 cores) distributes each layer's table shards.
"""
import os
import math
import time
from contextlib import ExitStack

import numpy as np

import concourse.bass as bass
import concourse.bacc as bacc
import concourse.mybir as mybir
import concourse.tile as tile
import concourse.bass_utils as bass_utils
import concourse.bass2jax as b2j

# ---------------------------------------------------------------- walrus flags
_orig_get_walrus_args = bass_utils.get_walrus_args


def _patched_walrus_args(*a, **kw):
    return _orig_get_walrus_args(*a, **kw) + [
        "--dge-levels=io,spill_reload,scalar_dynamic_offset,"
        "vector_dynamic_offsets,dynamic_size,transpose"]


bass_utils.get_walrus_args = _patched_walrus_args

P = 128
NCORES = 8
DIN = 128
DH = 64          # hidden feats (layer1 out)
DC = 40          # classes (layer2 out)
NEG = 0.2
CLASSES = [1, 2, 4, 8, 16, 32, 64, 128]
F32 = mybir.dt.float32


# ================================================================ host prep
def householder(a, n, col):
    """H (n x n) symmetric orthogonal with H @ unit(a) = e_col."""
    v = np.zeros(n, np.float64)
    v[: len(a)] = a
    na = np.linalg.norm(v)
    vu = v / na
    e = np.zeros(n, np.float64)
    e[col] = 1.0
    w = vu - e
    nw = np.linalg.norm(w)
    if nw < 1e-12:
        return np.eye(n), na
    w = w / nw
    H = np.eye(n) - 2.0 * np.outer(w, w)
    assert abs((H @ vu)[col] - 1.0) < 1e-9
    return H, na


class Plan:
    pass


def round_d(x):
    """Window degree = exact max degree (phase-variant S handles any d)."""
    return max(1, x)


def host_plan(src, dst, n_nodes):
    """Sorted-degree windows; per-window degree = max deg rounded to x4."""
    deg = np.bincount(dst, minlength=n_nodes)
    assert deg.max() <= 128, f"max degree {deg.max()} > 128 unsupported"
    plan = Plan()
    order = np.argsort(-deg, kind="stable")  # descending degree
    per_core = math.ceil(n_nodes / NCORES)
    F = math.ceil((per_core + 1) / P)  # +1 ensures >=1 dummy rank per core
    R = F * P
    rank_of = np.full(n_nodes, -1, np.int64)
    core_of = np.full(n_nodes, -1, np.int64)
    ranks = [np.full(R, -1, np.int64) for _ in range(NCORES)]
    for i, n in enumerate(order):
        c = i % NCORES
        r = i // NCORES
        ranks[c][r] = n
        rank_of[n] = r
        core_of[n] = c
    plan.R = R
    plan.ranks = ranks
    plan.rank_of = rank_of
    plan.core_of = core_of
    plan.zero_rank = R - 1
    for c in range(NCORES):
        assert ranks[c][plan.zero_rank] == -1, "zero rank must be a dummy"
    # per-window degree = global max over the 8 cores' 128-rank slices
    windows = []
    for w in range(F):
        dmax = 1
        for c in range(NCORES):
            rk = ranks[c][w * P:(w + 1) * P]
            v = rk >= 0
            if v.any():
                dmax = max(dmax, int(deg[rk[v]].max()))
        windows.append(round_d(dmax))
    plan.windows = windows
    NCHUNK = sum(windows)
    plan.NCHUNK = NCHUNK
    # adjacency
    adj = [[] for _ in range(n_nodes)]
    for e in range(len(src)):
        adj[dst[e]].append(src[e])
    gidx = []
    for c in range(NCORES):
        gi = np.zeros((P, NCHUNK), np.int32)
        zero_g = c * R + plan.zero_rank
        gi[:] = zero_g
        chunk0 = 0
        for w, d in enumerate(windows):
            for o in range(P):
                node = ranks[c][w * P + o]
                if node < 0:
                    continue
                for j, sn in enumerate(adj[node]):
                    e_local = o * d + j
                    gi[e_local % P, chunk0 + e_local // P] = \
                        core_of[sn] * R + rank_of[sn]
            chunk0 += d
        gidx.append(gi)
    plan.gidx = gidx
    return plan


def host_consts(W1, al1, ar1, b1, W2, al2, ar2, b2):
    H1, na1 = householder(np.asarray(al1, np.float64), DH, DH - 1)
    H2, na2 = householder(np.asarray(al2, np.float64), DC, DC - 1)
    D1 = np.eye(DH)
    D1[DH - 1, DH - 1] = na1
    D2 = np.eye(DC)
    D2[DC - 1, DC - 1] = na2
    W1f = np.asarray(W1, np.float64)
    W2f = np.asarray(W2, np.float64)
    waug1 = np.zeros((DIN, 65), np.float32)
    waug1[:, :DH] = (W1f @ H1 @ D1).astype(np.float32)
    waug1[:, 64] = (W1f @ np.asarray(ar1, np.float64)).astype(np.float32)
    # layer2: table row cols 0..39 = feat2 @ H2 D2, cols 40..63 zero
    waug2 = np.zeros((P, 65), np.float32)  # padded K=128 (rows 64.. zero)
    waug2[:DH, :DC] = (W2f @ H2 @ D2).astype(np.float32)
    waug2[:DH, 64] = (W2f @ np.asarray(ar2, np.float64)).astype(np.float32)
    # unrotate: g @ (D^-1 H) ; aug passthrough of s at col DH/DC
    un1 = np.zeros((P, 65), np.float32)
    un1[:DH, :DH] = (np.linalg.inv(D1) @ H1).astype(np.float32)
    un1[64, 64] = 1.0
    un2 = np.zeros((P, DC + 1), np.float32)
    un2[:DC, :DC] = (np.linalg.inv(D2) @ H2).astype(np.float32)
    un2[64, DC] = 1.0
    ident = np.eye(P, dtype=np.float32)
    b1rep = np.tile(np.asarray(b1, np.float32)[None, :], (P, 1))
    b2rep = np.tile(np.asarray(b2, np.float32)[None, :], (P, 1))
    return dict(waug1=waug1, waug2=waug2, un1=un1, un2=un2,
                ident=ident, b1rep=b1rep, b2rep=b2rep)


def host_sds(plan):
    """Phase-variant segment matrices, deduped by (d, phase)."""
    pairs = []
    seen = set()
    for d in plan.windows:
        for k in range(d):
            phi = (k * P) % d
            if (d, phi) not in seen:
                seen.add((d, phi))
                pairs.append((d, phi))
    sdcol = {}
    cols = []
    col = 0
    for (d, phi) in pairs:
        ncols = (phi + P - 1) // d + 1
        m = np.zeros((P, ncols), np.float32)
        for p in range(P):
            m[p, (phi + p) // d] = 1.0
        sdcol[(d, phi)] = (col, ncols)
        cols.append(m)
        col += ncols
    return np.concatenate(cols, axis=1), sdcol


# ================================================================ device build
def build_nc(plan, sds_arr, sdcol, debug=False):
    R = plan.R
    F = R // P
    NCH = plan.NCHUNK
    GRP = 16  # chunks per gather group tile

    nc = bacc.Bacc("TRN2", target_bir_lowering=False, debug=False,
                   num_devices=NCORES)
    xT = nc.declare_dram_parameter("xT", [DIN, R], F32, isOutput=False)
    gidx = nc.declare_dram_parameter("gidx", [P, NCH], mybir.dt.int32, isOutput=False)
    waug1 = nc.declare_dram_parameter("waug1", [DIN, 65], F32, isOutput=False)
    waug2 = nc.declare_dram_parameter("waug2", [P, 65], F32, isOutput=False)
    un1 = nc.declare_dram_parameter("un1", [P, 65], F32, isOutput=False)
    un2 = nc.declare_dram_parameter("un2", [P, DC + 1], F32, isOutput=False)
    sdsP = nc.declare_dram_parameter("sds", [P, sds_arr.shape[1]], F32, isOutput=False)
    identP = nc.declare_dram_parameter("ident", [P, P], F32, isOutput=False)
    b1repP = nc.declare_dram_parameter("b1rep", [P, DH], F32, isOutput=False)
    b2repP = nc.declare_dram_parameter("b2rep", [P, DC], F32, isOutput=False)
    outP = nc.declare_dram_parameter("out", [R, DC], F32, isOutput=True)
    if debug:
        dbg_t1 = nc.declare_dram_parameter("dbg_t1", [R, 64], F32, isOutput=True)
        dbg_er1 = nc.declare_dram_parameter("dbg_er1", [P, R // P], F32, isOutput=True)
        dbg_h = nc.declare_dram_parameter("dbg_h", [P, (R // P) * DH], F32, isOutput=True)

    # internal DRAM
    t1_shard = nc.dram_tensor("t1_shard", [R, 64], F32)
    t2_shard = nc.dram_tensor("t2_shard", [R, 64], F32)
    t1_full = nc.dram_tensor("t1_full", [NCORES * R, 64], F32, addr_space="Shared")
    t2_full = nc.dram_tensor("t2_full", [NCORES * R, 64], F32, addr_space="Shared")
    er1_d = nc.dram_tensor("er1_d", [R], F32)
    er2_d = nc.dram_tensor("er2_d", [R], F32)

    rg = [list(range(NCORES))]

    with ExitStack() as ctx:
        tc = ctx.enter_context(tile.TileContext(nc))
        cons = ctx.enter_context(tc.tile_pool(name="cons", bufs=1))
        big = ctx.enter_context(tc.tile_pool(name="big", bufs=1))
        sb = ctx.enter_context(tc.tile_pool(name="sb", bufs=3))
        gp = ctx.enter_context(tc.tile_pool(name="gp", bufs=8))
        ps = ctx.enter_context(tc.tile_pool(name="ps", bufs=2, space="PSUM"))

        # ---- constants
        waug1_t = cons.tile([DIN, 65], F32)
        nc.sync.dma_start(out=waug1_t[:], in_=waug1[:, :])
        waug2_t = cons.tile([P, 65], F32)
        nc.sync.dma_start(out=waug2_t[:], in_=waug2[:, :])
        un1_t = cons.tile([P, 65], F32)
        nc.sync.dma_start(out=un1_t[:], in_=un1[:, :])
        un2_t = cons.tile([P, DC + 1], F32)
        nc.sync.dma_start(out=un2_t[:], in_=un2[:, :])
        sds_t = cons.tile([P, sds_arr.shape[1]], F32)
        nc.sync.dma_start(out=sds_t[:], in_=sdsP[:, :])
        ident_t = cons.tile([P, P], F32)
        nc.sync.dma_start(out=ident_t[:], in_=identP[:, :])
        b1_t = cons.tile([P, DH], F32)
        nc.sync.dma_start(out=b1_t[:], in_=b1repP[:, :])
        b2_t = cons.tile([P, DC], F32)
        nc.sync.dma_start(out=b2_t[:], in_=b2repP[:, :])
        gidx_t = big.tile([P, NCH], mybir.dt.int32)
        nc.sync.dma_start(out=gidx_t[:], in_=gidx[:, :])
        negrow = cons.tile([1, 64], F32)
        nc.vector.memset(negrow[:], -1e30)

        # ---- layer1 table build
        xT_t = big.tile([DIN, R], F32)
        nc.sync.dma_start(out=xT_t[:], in_=xT[:, :])
        er1_slab = big.tile([P, F], F32)
        for f in range(F):
            pb = ps.tile([P, 65], F32, tag="bld")
            nc.tensor.matmul(pb[:], xT_t[:, f * P:(f + 1) * P], waug1_t[:],
                             start=True, stop=True)
            tb = sb.tile([P, 65], F32, tag="tb")
            nc.vector.tensor_copy(tb[:], pb[:])
            nc.scalar.dma_start(out=t1_shard[f * P:(f + 1) * P, :], in_=tb[:, 0:64])
            nc.vector.tensor_copy(er1_slab[:, f:f + 1], tb[:, 64:65])
        zr = plan.zero_rank
        nc.scalar.dma_start(out=t1_shard[zr:zr + 1, :], in_=negrow[:])
        er1_ap = bass.AP(tensor=er1_d, offset=0, ap=[[1, P], [P, F]])
        nc.sync.dma_start(out=er1_ap, in_=er1_slab[:])
        nc.gpsimd.collective_compute(
            "AllGather", mybir.AluOpType.bypass, replica_groups=rg,
            ins=[t1_shard[:, :].opt()], outs=[t1_full[:, :].opt()])

        # ---- edge phase helper
        h_slab = big.tile([P, F * DH], F32)  # layer1 output (rank tiles)
        er2_slab = big.tile([P, F], F32)

        def edge_phase(layer):
            table = t1_full if layer == 1 else t2_full
            er_d = er1_d if layer == 1 else er2_d
            un_t = un1_t if layer == 1 else un2_t
            ncols = 65 if layer == 1 else DC + 1
            elcol = 63 if layer == 1 else 39
            chunk0 = 0
            base = 0
            for w, d in enumerate(plan.windows):
                pseg = ps.tile([65, P], F32, tag="seg")
                ngrp = (d + GRP - 1) // GRP
                for g in range(ngrp):
                    kg = min(GRP, d - g * GRP)
                    c0 = chunk0 + g * GRP
                    gw = gp.tile([P, GRP * 64], F32, tag="gw")
                    for k in range(kg):
                        nc.gpsimd.indirect_dma_start(
                            out=gw[:, k * 64:(k + 1) * 64], out_offset=None,
                            in_=table[:, :],
                            in_offset=bass.IndirectOffsetOnAxis(
                                ap=gidx_t[:, c0 + k:c0 + k + 1], axis=0))
                    # er broadcast [128, kg]: partition p of chunk kk gets
                    # er[base + (kk*128 + p)//d]  (runs of d, phase phi)
                    erb = gp.tile([P, GRP], F32, tag="erb")
                    for k in range(kg):
                        kk = g * GRP + k
                        n0 = (kk * P) // d
                        phi = (kk * P) % d
                        pos = 0
                        n = n0
                        if phi > 0:
                            head = d - phi
                            nc.sync.dma_start(
                                out=erb[pos:pos + head, k:k + 1],
                                in_=bass.AP(tensor=er_d, offset=base + n,
                                            ap=[[0 if head > 1 else 1, head]]))
                            pos += head
                            n += 1
                        nfull = (P - pos) // d
                        if nfull > 0:
                            ap_nf = ([[1, nfull]] if d == 1 else
                                     [[1, nfull], [0, d]])
                            nc.sync.dma_start(
                                out=erb[pos:pos + nfull * d, k:k + 1],
                                in_=bass.AP(tensor=er_d, offset=base + n,
                                            ap=ap_nf))
                            pos += nfull * d
                            n += nfull
                        tail = P - pos
                        if tail > 0:
                            nc.sync.dma_start(
                                out=erb[pos:, k:k + 1],
                                in_=bass.AP(tensor=er_d, offset=base + n,
                                            ap=[[0 if tail > 1 else 1, tail]]))
                    # w = exp(lrelu(el + er))
                    wv = gp.tile([P, GRP], F32, tag="wv")
                    el_ap = gw[:].rearrange("p (k e) -> p k e", e=64)[:, :kg, elcol]
                    nc.vector.tensor_tensor(out=wv[:, :kg], in0=el_ap,
                                            in1=erb[:, :kg], op=mybir.AluOpType.add)
                    wv2 = gp.tile([P, GRP], F32, tag="wv2")
                    nc.vector.tensor_scalar(out=wv2[:, :kg], in0=wv[:, :kg],
                                            scalar1=NEG, scalar2=None,
                                            op0=mybir.AluOpType.mult)
                    nc.vector.tensor_tensor(out=wv[:, :kg], in0=wv[:, :kg],
                                            in1=wv2[:, :kg], op=mybir.AluOpType.max)
                    nc.scalar.activation(wv[:, :kg], wv[:, :kg],
                                         mybir.ActivationFunctionType.Exp)
                    # R rows
                    rr = gp.tile([P, GRP * 65], F32, tag="rr")
                    r3 = rr[:].rearrange("p (k e) -> p k e", e=65)
                    g3 = gw[:].rearrange("p (k e) -> p k e", e=64)
                    wb = wv[:, :kg].to_broadcast([P, kg, 64])
                    nc.vector.tensor_tensor(out=r3[:, :kg, 0:64], in0=g3[:, :kg, :],
                                            in1=wb, op=mybir.AluOpType.mult)
                    nc.vector.tensor_copy(r3[:, :kg, 64], wv[:, :kg])
                    for k in range(kg):
                        kk = g * GRP + k
                        n0 = (kk * P) // d
                        phi = (kk * P) % d
                        scol, ncols_s = sdcol[(d, phi)]
                        nc.tensor.matmul(
                            pseg[:, n0:n0 + ncols_s],
                            rr[:, k * 65:(k + 1) * 65],
                            sds_t[:, scol:scol + ncols_s],
                            start=(kk == 0), stop=(kk == d - 1))
                # ---- flush window w (ranks base..base+128)
                rawT = sb.tile([P, P], F32, tag="rawT")
                nc.vector.memset(rawT[64:, :], 0.0)
                nc.vector.tensor_copy(rawT[0:65, :], pseg[:])
                pun = ps.tile([P, 65], F32, tag="un")
                nc.tensor.matmul(pun[:, :ncols], rawT[:], un_t[:, :ncols],
                                 start=True, stop=True)
                # normalize + bias
                nd = DH if layer == 1 else DC
                rec = sb.tile([P, 1], F32, tag="rec")
                nc.vector.tensor_scalar(out=rec[:], in0=pun[:, nd:nd + 1],
                                        scalar1=1e-30, scalar2=None,
                                        op0=mybir.AluOpType.add)
                nc.vector.reciprocal(rec[:], rec[:])
                hv = sb.tile([P, nd], F32, tag="hv")
                nc.vector.tensor_scalar(out=hv[:], in0=pun[:, 0:nd], scalar1=rec[:],
                                        scalar2=None, op0=mybir.AluOpType.mult)
                nc.vector.tensor_tensor(out=hv[:], in0=hv[:],
                                        in1=(b1_t if layer == 1 else b2_t)[:],
                                        op=mybir.AluOpType.add)
                f = base // P
                if layer == 1:
                    # ELU: h = max(h, exp(min(h,0)) - 1)
                    t1t = sb.tile([P, DH], F32, tag="t1t")
                    nc.vector.tensor_scalar(out=t1t[:], in0=hv[:], scalar1=0.0,
                                            scalar2=None, op0=mybir.AluOpType.min)
                    nc.scalar.activation(t1t[:], t1t[:],
                                         mybir.ActivationFunctionType.Exp)
                    nc.vector.tensor_scalar(out=t1t[:], in0=t1t[:], scalar1=1.0,
                                            scalar2=None, op0=mybir.AluOpType.subtract)
                    nc.vector.tensor_tensor(out=hv[:], in0=hv[:], in1=t1t[:],
                                            op=mybir.AluOpType.max)
                    nc.vector.tensor_copy(h_slab[:, f * DH:(f + 1) * DH], hv[:])
                    # layer2 table row for this tile
                    hpad = sb.tile([P, P], F32, tag="hpad")
                    nc.vector.memset(hpad[:, DH:], 0.0)
                    nc.vector.tensor_copy(hpad[:, 0:DH], hv[:])
                    phT = ps.tile([P, P], F32, tag="tr")
                    nc.tensor.transpose(phT[:], hpad[:], ident_t[:])
                    hT = sb.tile([P, P], F32, tag="hT")
                    nc.vector.tensor_copy(hT[:], phT[:])
                    pb2 = ps.tile([P, 65], F32, tag="bld")
                    nc.tensor.matmul(pb2[:], hT[:], waug2_t[:], start=True, stop=True)
                    t2b = sb.tile([P, 65], F32, tag="tb")
                    nc.vector.tensor_copy(t2b[:], pb2[:])
                    nc.scalar.dma_start(out=t2_shard[f * P:(f + 1) * P, :],
                                        in_=t2b[:, 0:64])
                    nc.vector.tensor_copy(er2_slab[:, f:f + 1], t2b[:, 64:65])
                else:
                    nc.scalar.dma_start(out=outP[f * P:(f + 1) * P, :], in_=hv[:])
                chunk0 += d
                base += P

        edge_phase(1)
        if debug:
            nc.sync.dma_start(out=dbg_t1[:, :], in_=t1_shard[:, :])
            nc.sync.dma_start(out=dbg_er1[:, :], in_=er1_slab[:])
            nc.sync.dma_start(out=dbg_h[:, :], in_=h_slab[:])
        nc.scalar.dma_start(out=t2_shard[plan.zero_rank:plan.zero_rank + 1, :],
                            in_=negrow[:])
        er2_ap = bass.AP(tensor=er2_d, offset=0, ap=[[1, P], [P, F]])
        nc.sync.dma_start(out=er2_ap, in_=er2_slab[:])
        nc.gpsimd.collective_compute(
            "AllGather", mybir.AluOpType.bypass, replica_groups=rg,
            ins=[t2_shard[:, :].opt()], outs=[t2_full[:, :].opt()])
        edge_phase(2)

    nc.compile()
    return nc


# ================================================================ runner
class BassRunner:
    def __init__(self, nc, n_cores=NCORES):
        import jax
        from jax.experimental.shard_map import shard_map
        from jax.sharding import Mesh, PartitionSpec
        b2j.install_neuronx_cc_hook()
        self.jax = jax
        self.nc = nc
        self.n_cores = n_cores
        pname = nc.partition_id_tensor.name if nc.partition_id_tensor else None
        in_names, out_names, out_avals, zero_outs = [], [], [], []
        for alloc in nc.m.functions[0].allocations:
            if not isinstance(alloc, mybir.MemoryLocationSet):
                continue
            name = alloc.memorylocations[0].name
            if alloc.kind == "ExternalInput":
                if name != pname:
                    in_names.append(name)
            elif alloc.kind == "ExternalOutput":
                out_names.append(name)
                shape = tuple(alloc.tensor_shape)
                dtype = mybir.dt.np(alloc.dtype)
                out_avals.append(jax.core.ShapedArray(shape, dtype))
                zero_outs.append(np.zeros(shape, dtype))
        self.in_names, self.out_names = in_names, out_names
        self.out_avals, self.zero_outs = out_avals, zero_outs
        all_in = list(in_names) + list(out_names)
        if pname is not None:
            all_in.append(pname)

        def _body(*args):
            operands = list(args)
            if pname is not None:
                operands.append(b2j.partition_id_tensor())
            return tuple(b2j._bass_exec_p.bind(
                *operands, out_avals=tuple(out_avals), in_names=tuple(all_in),
                out_names=tuple(out_names), lowering_input_output_aliases=(),
                sim_require_finite=False, sim_require_nnan=False, nc=nc))

        devices = jax.devices()[:n_cores]
        self.mesh = Mesh(np.asarray(devices), ("core",))
        nio = len(in_names) + len(out_names)
        self._fn = jax.jit(
            shard_map(_body, mesh=self.mesh,
                      in_specs=(PartitionSpec("core"),) * nio,
                      out_specs=(PartitionSpec("core"),) * len(out_names),
                      check_rep=False),
            keep_unused=True)
        self._dev_in = None

    def put_inputs(self, in_maps):
        jax = self.jax
        from jax.sharding import PartitionSpec
        sharding = jax.sharding.NamedSharding(self.mesh, PartitionSpec("core"))
        concat = [np.concatenate([np.asarray(in_maps[c][n])
                                  for c in range(self.n_cores)], axis=0)
                  for n in self.in_names]
        zeros = [np.zeros((self.n_cores * z.shape[0], *z.shape[1:]), z.dtype)
                 for z in self.zero_outs]
        self._dev_in = [jax.device_put(a, sharding) for a in concat + zeros]
        jax.block_until_ready(self._dev_in)

    def run(self):
        outs = self._fn(*self._dev_in)
        self.jax.block_until_ready(outs)
        return outs

    def results(self, outs):
        res = []
        for c in range(self.n_cores):
            d = {}
            for i, name in enumerate(self.out_names):
                d[name] = np.asarray(outs[i]).reshape(
                    self.n_cores, *self.out_avals[i].shape)[c]
            res.append(d)
        return res


# ================================================================ entry point
_CACHE = {}


def kernel(in_feat, src, dst, W1, al1, ar1, b1, W2, al2, ar2, b2,
           _time_out=None):
    in_feat = np.asarray(in_feat)
    src = np.asarray(src)
    dst = np.asarray(dst)
    n_nodes = in_feat.shape[0]
    key = (n_nodes, len(src), int(src[0]), int(dst[0]), int(src[-1]))
    if key not in _CACHE:
        plan = host_plan(src, dst, n_nodes)
        sds_arr, sdcol = host_sds(plan)
        nc = build_nc(plan, sds_arr, sdcol)
        runner = BassRunner(nc)
        _CACHE[key] = (plan, sds_arr, runner)
    else:
        plan, sds_arr, runner = _CACHE[key]
    consts = host_consts(W1, al1, ar1, b1, W2, al2, ar2, b2)
    consts["sds"] = sds_arr

    in_maps = []
    for c in range(NCORES):
        xTc = np.zeros((DIN, plan.R), np.float32)
        rk = plan.ranks[c]
        valid = rk >= 0
        xTc[:, valid] = np.asarray(in_feat, np.float32)[rk[valid]].T
        m = {"xT": xTc, "gidx": plan.gidx[c]}
        for k in ("waug1", "waug2", "un1", "un2", "sds", "ident", "b1rep", "b2rep"):
            m[k] = consts[k]
        in_maps.append(m)
    runner.put_inputs(in_maps)
    t0 = time.perf_counter()
    outs = runner.run()
    wall = time.perf_counter() - t0
    if _time_out is not None:
        _time_out.append(wall)
        _time_out.append(runner)
    res = runner.results(outs)
    out_full = np.zeros((n_nodes, DC), np.float32)
    for c in range(NCORES):
        rk = plan.ranks[c]
        valid = rk >= 0
        out_full[rk[valid]] = res[c]["out"][valid]
    return out_full


# revision 19
# speedup vs baseline: 1.2894x; 1.2894x over previous
"""GAT (2-layer) Trainium2 Bass kernel — 8-core SPMD.

Strategy (dst-partitioned, degree-regular):
  - Host: nodes bucketed by in-degree (padded to pow2 classes d in {1..128}),
    dealt round-robin to 8 cores, padded so all cores share one class profile.
    Node "rank" space (per core) replaces node ids everywhere on device.
  - Table trick: gather table row = feat @ (W·H·D) where H = Householder
    mapping attn_l -> e_col, D scales that col by ||attn_l||. So the per-edge
    gather brings both the features (rotated) and el = feat@attn_l in ONE row.
    er = feat@(W@attn_r) computed per node at table build.
  - Edge phase: per 128-node window of one degree class d: gather k=d chunks
    of 128 edge-rows (indirect DMA, edge e at [e%128, e//128]); er broadcast
    per window via a step-0 strided DMA; w = exp(leakyrelu(el+er));
    segment-sum via ONE constant matmul per chunk (S_d = I_{128/d} (x) 1_d),
    PSUM node-major [128, 65] = [sum w*feat' | sum w].
  - Flush per window: transpose (PE), unrotate via (H D^-1) matmul, divide by
    s, +bias, (ELU) -> h tile; layer-2 table built per tile on the fly.
  - AllGather (8 cores) distributes each layer's table shards.
"""
import os
import math
import time
from contextlib import ExitStack

import numpy as np

import concourse.bass as bass
import concourse.bacc as bacc
import concourse.mybir as mybir
import concourse.tile as tile
import concourse.bass_utils as bass_utils
import concourse.bass2jax as b2j

# ---------------------------------------------------------------- walrus flags
_orig_get_walrus_args = bass_utils.get_walrus_args


def _patched_walrus_args(*a, **kw):
    return _orig_get_walrus_args(*a, **kw) + [
        "--dge-levels=io,spill_reload,scalar_dynamic_offset,"
        "vector_dynamic_offsets,dynamic_size,transpose"]


bass_utils.get_walrus_args = _patched_walrus_args

P = 128
NCORES = 8
DIN = 128
DH = 64          # hidden feats (layer1 out)
DC = 40          # classes (layer2 out)
NEG = 0.2
CLASSES = [1, 2, 4, 8, 16, 32, 64, 128]
F32 = mybir.dt.float32


# ================================================================ host prep
def householder(a, n, col):
    """H (n x n) symmetric orthogonal with H @ unit(a) = e_col."""
    v = np.zeros(n, np.float64)
    v[: len(a)] = a
    na = np.linalg.norm(v)
    vu = v / na
    e = np.zeros(n, np.float64)
    e[col] = 1.0
    w = vu - e
    nw = np.linalg.norm(w)
    if nw < 1e-12:
        return np.eye(n), na
    w = w / nw
    H = np.eye(n) - 2.0 * np.outer(w, w)
    assert abs((H @ vu)[col] - 1.0) < 1e-9
    return H, na


class Plan:
    pass


def round_d(x):
    """Window degree = exact max degree (phase-variant S handles any d)."""
    return max(1, x)


def host_plan(src, dst, n_nodes):
    """Sorted-degree windows; per-window degree = max deg rounded to x4."""
    deg = np.bincount(dst, minlength=n_nodes)
    assert deg.max() <= 128, f"max degree {deg.max()} > 128 unsupported"
    plan = Plan()
    order = np.argsort(-deg, kind="stable")  # descending degree
    per_core = math.ceil(n_nodes / NCORES)
    F = math.ceil((per_core + 1) / P)  # +1 ensures >=1 dummy rank per core
    R = F * P
    rank_of = np.full(n_nodes, -1, np.int64)
    core_of = np.full(n_nodes, -1, np.int64)
    ranks = [np.full(R, -1, np.int64) for _ in range(NCORES)]
    for i, n in enumerate(order):
        c = i % NCORES
        r = i // NCORES
        ranks[c][r] = n
        rank_of[n] = r
        core_of[n] = c
    plan.R = R
    plan.ranks = ranks
    plan.rank_of = rank_of
    plan.core_of = core_of
    plan.zero_rank = R - 1
    for c in range(NCORES):
        assert ranks[c][plan.zero_rank] == -1, "zero rank must be a dummy"
    # per-window degree = global max over the 8 cores' 128-rank slices
    windows = []
    for w in range(F):
        dmax = 1
        for c in range(NCORES):
            rk = ranks[c][w * P:(w + 1) * P]
            v = rk >= 0
            if v.any():
                dmax = max(dmax, int(deg[rk[v]].max()))
        windows.append(round_d(dmax))
    plan.windows = windows
    NCHUNK = sum(windows)
    plan.NCHUNK = NCHUNK
    # adjacency
    adj = [[] for _ in range(n_nodes)]
    for e in range(len(src)):
        adj[dst[e]].append(src[e])
    gidx = []
    for c in range(NCORES):
        gi = np.zeros((P, NCHUNK), np.int32)
        zero_g = c * R + plan.zero_rank
        gi[:] = zero_g
        chunk0 = 0
        for w, d in enumerate(windows):
            for o in range(P):
                node = ranks[c][w * P + o]
                if node < 0:
                    continue
                for j, sn in enumerate(adj[node]):
                    e_local = o * d + j
                    gi[e_local % P, chunk0 + e_local // P] = \
                        core_of[sn] * R + rank_of[sn]
            chunk0 += d
        gidx.append(gi)
    plan.gidx = gidx
    return plan


def host_consts(W1, al1, ar1, b1, W2, al2, ar2, b2):
    H1, na1 = householder(np.asarray(al1, np.float64), DH, DH - 1)
    H2, na2 = householder(np.asarray(al2, np.float64), DC, DC - 1)
    D1 = np.eye(DH)
    D1[DH - 1, DH - 1] = na1
    D2 = np.eye(DC)
    D2[DC - 1, DC - 1] = na2
    W1f = np.asarray(W1, np.float64)
    W2f = np.asarray(W2, np.float64)
    waug1 = np.zeros((DIN, 65), np.float32)
    waug1[:, :DH] = (W1f @ H1 @ D1).astype(np.float32)
    waug1[:, 64] = (W1f @ np.asarray(ar1, np.float64)).astype(np.float32)
    # layer2: table row cols 0..39 = feat2 @ H2 D2, cols 40..63 zero
    waug2 = np.zeros((P, 65), np.float32)  # padded K=128 (rows 64.. zero)
    waug2[:DH, :DC] = (W2f @ H2 @ D2).astype(np.float32)
    waug2[:DH, 64] = (W2f @ np.asarray(ar2, np.float64)).astype(np.float32)
    # unrotate: g @ (D^-1 H) ; aug passthrough of s at col DH/DC
    un1 = np.zeros((P, 65), np.float32)
    un1[:DH, :DH] = (np.linalg.inv(D1) @ H1).astype(np.float32)
    un1[64, 64] = 1.0
    un2 = np.zeros((P, DC + 1), np.float32)
    un2[:DC, :DC] = (np.linalg.inv(D2) @ H2).astype(np.float32)
    un2[64, DC] = 1.0
    ident = np.eye(P, dtype=np.float32)
    b1rep = np.tile(np.asarray(b1, np.float32)[None, :], (P, 1))
    b2rep = np.tile(np.asarray(b2, np.float32)[None, :], (P, 1))
    return dict(waug1=waug1, waug2=waug2, un1=un1, un2=un2,
                ident=ident, b1rep=b1rep, b2rep=b2rep)


def host_sds(plan):
    """Phase-variant segment matrices, deduped by (d, phase)."""
    pairs = []
    seen = set()
    for d in plan.windows:
        for k in range(d):
            phi = (k * P) % d
            if (d, phi) not in seen:
                seen.add((d, phi))
                pairs.append((d, phi))
    sdcol = {}
    cols = []
    col = 0
    for (d, phi) in pairs:
        ncols = (phi + P - 1) // d + 1
        m = np.zeros((P, ncols), np.float32)
        for p in range(P):
            m[p, (phi + p) // d] = 1.0
        sdcol[(d, phi)] = (col, ncols)
        cols.append(m)
        col += ncols
    return np.concatenate(cols, axis=1), sdcol


# ================================================================ device build
def build_nc(plan, sds_arr, sdcol, debug=False):
    R = plan.R
    F = R // P
    NCH = plan.NCHUNK
    GRP = 16  # chunks per gather group tile

    nc = bacc.Bacc("TRN2", target_bir_lowering=False, debug=False,
                   num_devices=NCORES)
    xT = nc.declare_dram_parameter("xT", [DIN, R], F32, isOutput=False)
    gidx = nc.declare_dram_parameter("gidx", [P, NCH], mybir.dt.int32, isOutput=False)
    waug1 = nc.declare_dram_parameter("waug1", [DIN, 65], F32, isOutput=False)
    waug2 = nc.declare_dram_parameter("waug2", [P, 65], F32, isOutput=False)
    un1 = nc.declare_dram_parameter("un1", [P, 65], F32, isOutput=False)
    un2 = nc.declare_dram_parameter("un2", [P, DC + 1], F32, isOutput=False)
    sdsP = nc.declare_dram_parameter("sds", [P, sds_arr.shape[1]], F32, isOutput=False)
    identP = nc.declare_dram_parameter("ident", [P, P], F32, isOutput=False)
    b1repP = nc.declare_dram_parameter("b1rep", [P, DH], F32, isOutput=False)
    b2repP = nc.declare_dram_parameter("b2rep", [P, DC], F32, isOutput=False)
    outP = nc.declare_dram_parameter("out", [R, DC], F32, isOutput=True)
    if debug:
        dbg_t1 = nc.declare_dram_parameter("dbg_t1", [R, 64], F32, isOutput=True)
        dbg_er1 = nc.declare_dram_parameter("dbg_er1", [P, R // P], F32, isOutput=True)
        dbg_h = nc.declare_dram_parameter("dbg_h", [P, (R // P) * DH], F32, isOutput=True)

    # internal DRAM
    t1_shard = nc.dram_tensor("t1_shard", [R, 64], F32)
    t2_shard = nc.dram_tensor("t2_shard", [R, 64], F32)
    t1_full = nc.dram_tensor("t1_full", [NCORES * R, 64], F32, addr_space="Shared")
    t2_full = nc.dram_tensor("t2_full", [NCORES * R, 64], F32, addr_space="Shared")
    er1_d = nc.dram_tensor("er1_d", [R], F32)
    er2_d = nc.dram_tensor("er2_d", [R], F32)

    rg = [list(range(NCORES))]

    with ExitStack() as ctx:
        tc = ctx.enter_context(tile.TileContext(nc))
        cons = ctx.enter_context(tc.tile_pool(name="cons", bufs=1))
        big = ctx.enter_context(tc.tile_pool(name="big", bufs=1))
        sb = ctx.enter_context(tc.tile_pool(name="sb", bufs=3))
        gp = ctx.enter_context(tc.tile_pool(name="gp", bufs=8))
        ps = ctx.enter_context(tc.tile_pool(name="ps", bufs=2, space="PSUM"))

        # ---- constants
        waug1_t = cons.tile([DIN, 65], F32)
        nc.sync.dma_start(out=waug1_t[:], in_=waug1[:, :])
        waug2_t = cons.tile([P, 65], F32)
        nc.sync.dma_start(out=waug2_t[:], in_=waug2[:, :])
        un1_t = cons.tile([P, 65], F32)
        nc.sync.dma_start(out=un1_t[:], in_=un1[:, :])
        un2_t = cons.tile([P, DC + 1], F32)
        nc.sync.dma_start(out=un2_t[:], in_=un2[:, :])
        sds_t = cons.tile([P, sds_arr.shape[1]], F32)
        nc.sync.dma_start(out=sds_t[:], in_=sdsP[:, :])
        ident_t = cons.tile([P, P], F32)
        nc.sync.dma_start(out=ident_t[:], in_=identP[:, :])
        b1_t = cons.tile([P, DH], F32)
        nc.sync.dma_start(out=b1_t[:], in_=b1repP[:, :])
        b2_t = cons.tile([P, DC], F32)
        nc.sync.dma_start(out=b2_t[:], in_=b2repP[:, :])
        gidx_t = big.tile([P, NCH], mybir.dt.int32)
        nc.sync.dma_start(out=gidx_t[:], in_=gidx[:, :])
        negrow = cons.tile([1, 64], F32)
        nc.vector.memset(negrow[:], -1e30)

        # ---- layer1 table build
        xT_t = big.tile([DIN, R], F32)
        nc.sync.dma_start(out=xT_t[:], in_=xT[:, :])
        er1_slab = big.tile([P, F], F32)
        for f in range(F):
            pb = ps.tile([P, 65], F32, tag="bld")
            nc.tensor.matmul(pb[:], xT_t[:, f * P:(f + 1) * P], waug1_t[:],
                             start=True, stop=True)
            tb = sb.tile([P, 65], F32, tag="tb")
            nc.vector.tensor_copy(tb[:], pb[:])
            nc.scalar.dma_start(out=t1_shard[f * P:(f + 1) * P, :], in_=tb[:, 0:64])
            nc.vector.tensor_copy(er1_slab[:, f:f + 1], tb[:, 64:65])
        zr = plan.zero_rank
        nc.scalar.dma_start(out=t1_shard[zr:zr + 1, :], in_=negrow[:])
        er1_ap = bass.AP(tensor=er1_d, offset=0, ap=[[1, P], [P, F]])
        nc.sync.dma_start(out=er1_ap, in_=er1_slab[:])
        nc.gpsimd.collective_compute(
            "AllGather", mybir.AluOpType.bypass, replica_groups=rg,
            ins=[t1_shard[:, :].opt()], outs=[t1_full[:, :].opt()])

        # ---- edge phase helper
        h_slab = big.tile([P, F * DH], F32)  # layer1 output (rank tiles)
        er2_slab = big.tile([P, F], F32)

        def edge_phase(layer):
            table = t1_full if layer == 1 else t2_full
            er_d = er1_d if layer == 1 else er2_d
            un_t = un1_t if layer == 1 else un2_t
            ncols = 65 if layer == 1 else DC + 1
            elcol = 63 if layer == 1 else 39
            chunk0 = 0
            base = 0
            for w, d in enumerate(plan.windows):
                pseg = ps.tile([65, P], F32, tag="seg")
                ngrp = (d + GRP - 1) // GRP
                for g in range(ngrp):
                    kg = min(GRP, d - g * GRP)
                    c0 = chunk0 + g * GRP
                    gw = gp.tile([P, GRP * 64], F32, tag="gw")
                    for k in range(kg):
                        nc.gpsimd.indirect_dma_start(
                            out=gw[:, k * 64:(k + 1) * 64], out_offset=None,
                            in_=table[:, :],
                            in_offset=bass.IndirectOffsetOnAxis(
                                ap=gidx_t[:, c0 + k:c0 + k + 1], axis=0))
                    # er broadcast [128, kg]: partition p of chunk kk gets
                    # er[base + (kk*128 + p)//d]  (runs of d, phase phi)
                    erb = gp.tile([P, GRP], F32, tag="erb")
                    for k in range(kg):
                        kk = g * GRP + k
                        eng = nc.sync if (kk % 2 == 0) else nc.scalar
                        n0 = (kk * P) // d
                        phi = (kk * P) % d
                        pos = 0
                        n = n0
                        if phi > 0:
                            head = d - phi
                            eng.dma_start(
                                out=erb[pos:pos + head, k:k + 1],
                                in_=bass.AP(tensor=er_d, offset=base + n,
                                            ap=[[0 if head > 1 else 1, head]]))
                            pos += head
                            n += 1
                        nfull = (P - pos) // d
                        if nfull > 0:
                            ap_nf = ([[1, nfull]] if d == 1 else
                                     [[1, nfull], [0, d]])
                            eng.dma_start(
                                out=erb[pos:pos + nfull * d, k:k + 1],
                                in_=bass.AP(tensor=er_d, offset=base + n,
                                            ap=ap_nf))
                            pos += nfull * d
                            n += nfull
                        tail = P - pos
                        if tail > 0:
                            eng.dma_start(
                                out=erb[pos:, k:k + 1],
                                in_=bass.AP(tensor=er_d, offset=base + n,
                                            ap=[[0 if tail > 1 else 1, tail]]))
                    # w = exp(lrelu(el + er))
                    wv = gp.tile([P, GRP], F32, tag="wv")
                    el_ap = gw[:].rearrange("p (k e) -> p k e", e=64)[:, :kg, elcol]
                    nc.vector.tensor_tensor(out=wv[:, :kg], in0=el_ap,
                                            in1=erb[:, :kg], op=mybir.AluOpType.add)
                    wv2 = gp.tile([P, GRP], F32, tag="wv2")
                    nc.vector.tensor_scalar(out=wv2[:, :kg], in0=wv[:, :kg],
                                            scalar1=NEG, scalar2=None,
                                            op0=mybir.AluOpType.mult)
                    nc.vector.tensor_tensor(out=wv[:, :kg], in0=wv[:, :kg],
                                            in1=wv2[:, :kg], op=mybir.AluOpType.max)
                    nc.scalar.activation(wv[:, :kg], wv[:, :kg],
                                         mybir.ActivationFunctionType.Exp)
                    # R rows
                    rr = gp.tile([P, GRP * 65], F32, tag="rr")
                    r3 = rr[:].rearrange("p (k e) -> p k e", e=65)
                    g3 = gw[:].rearrange("p (k e) -> p k e", e=64)
                    wb = wv[:, :kg].to_broadcast([P, kg, 64])
                    nc.vector.tensor_tensor(out=r3[:, :kg, 0:64], in0=g3[:, :kg, :],
                                            in1=wb, op=mybir.AluOpType.mult)
                    nc.vector.tensor_copy(r3[:, :kg, 64], wv[:, :kg])
                    for k in range(kg):
                        kk = g * GRP + k
                        n0 = (kk * P) // d
                        phi = (kk * P) % d
                        scol, ncols_s = sdcol[(d, phi)]
                        nc.tensor.matmul(
                            pseg[:, n0:n0 + ncols_s],
                            rr[:, k * 65:(k + 1) * 65],
                            sds_t[:, scol:scol + ncols_s],
                            start=(kk == 0), stop=(kk == d - 1))
                # ---- flush window w (ranks base..base+128)
                rawT = sb.tile([P, P], F32, tag="rawT")
                nc.vector.memset(rawT[64:, :], 0.0)
                nc.vector.tensor_copy(rawT[0:65, :], pseg[:])
                pun = ps.tile([P, 65], F32, tag="un")
                nc.tensor.matmul(pun[:, :ncols], rawT[:], un_t[:, :ncols],
                                 start=True, stop=True)
                # normalize + bias
                nd = DH if layer == 1 else DC
                rec = sb.tile([P, 1], F32, tag="rec")
                nc.vector.tensor_scalar(out=rec[:], in0=pun[:, nd:nd + 1],
                                        scalar1=1e-30, scalar2=None,
                                        op0=mybir.AluOpType.add)
                nc.vector.reciprocal(rec[:], rec[:])
                hv = sb.tile([P, nd], F32, tag="hv")
                nc.vector.tensor_scalar(out=hv[:], in0=pun[:, 0:nd], scalar1=rec[:],
                                        scalar2=None, op0=mybir.AluOpType.mult)
                nc.vector.tensor_tensor(out=hv[:], in0=hv[:],
                                        in1=(b1_t if layer == 1 else b2_t)[:],
                                        op=mybir.AluOpType.add)
                f = base // P
                if layer == 1:
                    # ELU: h = max(h, exp(min(h,0)) - 1)
                    t1t = sb.tile([P, DH], F32, tag="t1t")
                    nc.vector.tensor_scalar(out=t1t[:], in0=hv[:], scalar1=0.0,
                                            scalar2=None, op0=mybir.AluOpType.min)
                    nc.scalar.activation(t1t[:], t1t[:],
                                         mybir.ActivationFunctionType.Exp)
                    nc.vector.tensor_scalar(out=t1t[:], in0=t1t[:], scalar1=1.0,
                                            scalar2=None, op0=mybir.AluOpType.subtract)
                    nc.vector.tensor_tensor(out=hv[:], in0=hv[:], in1=t1t[:],
                                            op=mybir.AluOpType.max)
                    nc.vector.tensor_copy(h_slab[:, f * DH:(f + 1) * DH], hv[:])
                    # layer2 table row for this tile
                    hpad = sb.tile([P, P], F32, tag="hpad")
                    nc.vector.memset(hpad[:, DH:], 0.0)
                    nc.vector.tensor_copy(hpad[:, 0:DH], hv[:])
                    phT = ps.tile([P, P], F32, tag="tr")
                    nc.tensor.transpose(phT[:], hpad[:], ident_t[:])
                    hT = sb.tile([P, P], F32, tag="hT")
                    nc.vector.tensor_copy(hT[:], phT[:])
                    pb2 = ps.tile([P, 65], F32, tag="bld")
                    nc.tensor.matmul(pb2[:], hT[:], waug2_t[:], start=True, stop=True)
                    t2b = sb.tile([P, 65], F32, tag="tb")
                    nc.vector.tensor_copy(t2b[:], pb2[:])
                    nc.scalar.dma_start(out=t2_shard[f * P:(f + 1) * P, :],
                                        in_=t2b[:, 0:64])
                    nc.vector.tensor_copy(er2_slab[:, f:f + 1], t2b[:, 64:65])
                else:
                    nc.scalar.dma_start(out=outP[f * P:(f + 1) * P, :], in_=hv[:])
                chunk0 += d
                base += P

        edge_phase(1)
        if debug:
            nc.sync.dma_start(out=dbg_t1[:, :], in_=t1_shard[:, :])
            nc.sync.dma_start(out=dbg_er1[:, :], in_=er1_slab[:])
            nc.sync.dma_start(out=dbg_h[:, :], in_=h_slab[:])
        nc.scalar.dma_start(out=t2_shard[plan.zero_rank:plan.zero_rank + 1, :],
                            in_=negrow[:])
        er2_ap = bass.AP(tensor=er2_d, offset=0, ap=[[1, P], [P, F]])
        nc.sync.dma_start(out=er2_ap, in_=er2_slab[:])
        nc.gpsimd.collective_compute(
            "AllGather", mybir.AluOpType.bypass, replica_groups=rg,
            ins=[t2_shard[:, :].opt()], outs=[t2_full[:, :].opt()])
        edge_phase(2)

    nc.compile()
    return nc


# ================================================================ runner
class BassRunner:
    def __init__(self, nc, n_cores=NCORES):
        import jax
        from jax.experimental.shard_map import shard_map
        from jax.sharding import Mesh, PartitionSpec
        b2j.install_neuronx_cc_hook()
        self.jax = jax
        self.nc = nc
        self.n_cores = n_cores
        pname = nc.partition_id_tensor.name if nc.partition_id_tensor else None
        in_names, out_names, out_avals, zero_outs = [], [], [], []
        for alloc in nc.m.functions[0].allocations:
            if not isinstance(alloc, mybir.MemoryLocationSet):
                continue
            name = alloc.memorylocations[0].name
            if alloc.kind == "ExternalInput":
                if name != pname:
                    in_names.append(name)
            elif alloc.kind == "ExternalOutput":
                out_names.append(name)
                shape = tuple(alloc.tensor_shape)
                dtype = mybir.dt.np(alloc.dtype)
                out_avals.append(jax.core.ShapedArray(shape, dtype))
                zero_outs.append(np.zeros(shape, dtype))
        self.in_names, self.out_names = in_names, out_names
        self.out_avals, self.zero_outs = out_avals, zero_outs
        all_in = list(in_names) + list(out_names)
        if pname is not None:
            all_in.append(pname)

        def _body(*args):
            operands = list(args)
            if pname is not None:
                operands.append(b2j.partition_id_tensor())
            return tuple(b2j._bass_exec_p.bind(
                *operands, out_avals=tuple(out_avals), in_names=tuple(all_in),
                out_names=tuple(out_names), lowering_input_output_aliases=(),
                sim_require_finite=False, sim_require_nnan=False, nc=nc))

        devices = jax.devices()[:n_cores]
        self.mesh = Mesh(np.asarray(devices), ("core",))
        nio = len(in_names) + len(out_names)
        self._fn = jax.jit(
            shard_map(_body, mesh=self.mesh,
                      in_specs=(PartitionSpec("core"),) * nio,
                      out_specs=(PartitionSpec("core"),) * len(out_names),
                      check_rep=False),
            keep_unused=True)
        self._dev_in = None

    def put_inputs(self, in_maps):
        jax = self.jax
        from jax.sharding import PartitionSpec
        sharding = jax.sharding.NamedSharding(self.mesh, PartitionSpec("core"))
        concat = [np.concatenate([np.asarray(in_maps[c][n])
                                  for c in range(self.n_cores)], axis=0)
                  for n in self.in_names]
        zeros = [np.zeros((self.n_cores * z.shape[0], *z.shape[1:]), z.dtype)
                 for z in self.zero_outs]
        self._dev_in = [jax.device_put(a, sharding) for a in concat + zeros]
        jax.block_until_ready(self._dev_in)

    def run(self):
        outs = self._fn(*self._dev_in)
        self.jax.block_until_ready(outs)
        return outs

    def results(self, outs):
        res = []
        for c in range(self.n_cores):
            d = {}
            for i, name in enumerate(self.out_names):
                d[name] = np.asarray(outs[i]).reshape(
                    self.n_cores, *self.out_avals[i].shape)[c]
            res.append(d)
        return res


# ================================================================ entry point
_CACHE = {}


def kernel(in_feat, src, dst, W1, al1, ar1, b1, W2, al2, ar2, b2,
           _time_out=None):
    in_feat = np.asarray(in_feat)
    src = np.asarray(src)
    dst = np.asarray(dst)
    n_nodes = in_feat.shape[0]
    key = (n_nodes, len(src), int(src[0]), int(dst[0]), int(src[-1]))
    if key not in _CACHE:
        plan = host_plan(src, dst, n_nodes)
        sds_arr, sdcol = host_sds(plan)
        nc = build_nc(plan, sds_arr, sdcol)
        runner = BassRunner(nc)
        _CACHE[key] = (plan, sds_arr, runner)
    else:
        plan, sds_arr, runner = _CACHE[key]
    consts = host_consts(W1, al1, ar1, b1, W2, al2, ar2, b2)
    consts["sds"] = sds_arr

    in_maps = []
    for c in range(NCORES):
        xTc = np.zeros((DIN, plan.R), np.float32)
        rk = plan.ranks[c]
        valid = rk >= 0
        xTc[:, valid] = np.asarray(in_feat, np.float32)[rk[valid]].T
        m = {"xT": xTc, "gidx": plan.gidx[c]}
        for k in ("waug1", "waug2", "un1", "un2", "sds", "ident", "b1rep", "b2rep"):
            m[k] = consts[k]
        in_maps.append(m)
    runner.put_inputs(in_maps)
    t0 = time.perf_counter()
    outs = runner.run()
    wall = time.perf_counter() - t0
    if _time_out is not None:
        _time_out.append(wall)
        _time_out.append(runner)
    res = runner.results(outs)
    out_full = np.zeros((n_nodes, DC), np.float32)
    for c in range(NCORES):
        rk = plan.ranks[c]
        valid = rk >= 0
        out_full[rk[valid]] = res[c]["out"][valid]
    return out_full
